# revision 6
# baseline (speedup 1.0000x reference)
"""Trainium2 Bass kernel for nn_AffineExpert (diag + rank-R linear recurrence).

Math: s_{t+1} = a_t*s_t + u_t + U (g_t * (V^T s_t)),  s_0 = 0, output s_S.
  a = sigmoid(x@Wa^T + ba), g = x@Wg^T + bg, u = x@Wu^T + bu.

Strategy per core (data-parallel over batch, 2 rows/core on 8 cores):

  * All heavy projections (a, u, g) are fp16 PE matmuls with fp32 PSUM
    accumulation, N=512 time-tiles; inputs staged to fp16 on the host so
    DMA moves half the bytes and no on-device cast is needed.
  * The recurrence is linear in the rank-R channel q_t = g_t*(V^T s_t).
    Per 512-step chunk: z0 = diag-decay scan of (a, u) from the carried
    state (DVE tensor_tensor_scan), p0 = V^T z0, q = g * shift(p0), then
    one more scan of (a, U q) gives the low-rank correction.  One
    fixed-point pass (no inner correction iteration) keeps the final
    error ~1.5e-3, well under the 2e-2 gate.
  * Chunks chain through the carried fp16 state columns; q's first step
    uses p_first = V^T s_final of the previous chunk (computed by tiny
    N=1 matmuls on the updated state).
  * Software pipelining: projection matmuls of chunk c+1 are emitted
    interleaved with the recurrence phase of chunk c, so the PE always
    has dense independent work while the DVE runs the serial scan chain
    (keeps the PE HAM-warm at 2.4 GHz; the old version oscillated cold).
"""
import numpy as np

import concourse.bass as bass
import concourse.mybir as mybir
import concourse.tile as tile
from concourse import bacc
from concourse.bass_utils import run_bass_kernel_spmd

f32 = mybir.dt.float32
f16 = mybir.dt.float16
AF = mybir.ActivationFunctionType
OP = mybir.AluOpType

B, S, D, H, R = 16, 2048, 1024, 1024, 16
N_CORES = 8
B_CORE = B // N_CORES
CHUNK = 512


def build_kernel(B_core=B_CORE, S_=S, D_=D, H_=H, R_=R, C=CHUNK):
    KC, HC, NCH = D_ // 128, H_ // 128, S_ // C
    nc = bacc.Bacc("TRN2")

    # host-prepped, chunk-contiguous fp16 x: [NCH, B_core, KC, 128, C]
    xc = nc.dram_tensor("xc", [NCH, B_core, KC, 128, C], f16, kind="ExternalInput")
    waT = nc.dram_tensor("waT", [KC, 128, H_], f16, kind="ExternalInput")
    wuT = nc.dram_tensor("wuT", [KC, 128, H_], f16, kind="ExternalInput")
    wgT = nc.dram_tensor("wgT", [KC, 128, R_], f16, kind="ExternalInput")
    uT_d = nc.dram_tensor("uT", [R_, H_], f16, kind="ExternalInput")
    v_d = nc.dram_tensor("v", [128, HC, R_], f16, kind="ExternalInput")
    ba_d = nc.dram_tensor("ba", [128, HC], f32, kind="ExternalInput")
    bu_d = nc.dram_tensor("bu", [128, HC], f32, kind="ExternalInput")
    bg_d = nc.dram_tensor("bg", [R_, 1], f32, kind="ExternalInput")
    out_d = nc.dram_tensor("out", [B_core, H_], f16, kind="ExternalOutput")

    with tile.TileContext(nc) as tc:
        with tc.tile_pool(name="persist", bufs=1) as persist, \
             tc.tile_pool(name="xpool", bufs=2) as xpool, \
             tc.tile_pool(name="apool", bufs=2) as apool, \
             tc.tile_pool(name="upool", bufs=2) as upool, \
             tc.tile_pool(name="zpool", bufs=2) as zpool, \
             tc.tile_pool(name="spool", bufs=2) as spool, \
             tc.tile_pool(name="clpool", bufs=4) as clpool, \
             tc.tile_pool(name="ps_proj", bufs=3, space="PSUM") as ps_proj, \
             tc.tile_pool(name="ps_uq", bufs=2, space="PSUM") as ps_uq, \
             tc.tile_pool(name="ps_p", bufs=2, space="PSUM") as ps_p, \
             tc.tile_pool(name="ps_tiny", bufs=1, space="PSUM") as ps_tiny:

            # ---------- persistent staging ----------
            wa16 = persist.tile([128, KC, H_], f16)
            wu16 = persist.tile([128, KC, H_], f16)
            wg16 = persist.tile([128, KC, R_], f16)
            v16 = persist.tile([128, HC, R_], f16)
            u16T = persist.tile([R_, H_], f16)
            ba_t = persist.tile([128, HC], f32)
            bu_t = persist.tile([128, HC], f32)
            bg_t = persist.tile([R_, 1], f32)
            state16 = persist.tile([128, B_core * HC], f16)
            p_first = persist.tile([R_, B_core], f32)

            # staging spread across engine DGE queues so it parallelizes
            # (dma_start is only legal from sync/scalar/gpsimd)
            for kc in range(KC):
                nc.scalar.dma_start(wg16[:, kc, :], wgT[kc])
            for kc in range(KC):
                nc.gpsimd.dma_start(wa16[:, kc, :], waT[kc])
            for kc in range(KC):
                nc.scalar.dma_start(wu16[:, kc, :], wuT[kc])
            nc.gpsimd.dma_start(v16[:], v_d[:, :, :])
            nc.gpsimd.dma_start(u16T[:], uT_d[:, :])
            nc.scalar.dma_start(ba_t[:], ba_d[:, :])
            nc.gpsimd.dma_start(bu_t[:], bu_d[:, :])
            nc.scalar.dma_start(bg_t[:], bg_d[:, :])

            nc.vector.memset(state16[:], 0.0)
            nc.vector.memset(p_first[:], 0.0)

            # ---------- emission helpers ----------
            x16 = {}     # (row, kc) -> live x tile of the chunk being projected
            a16 = {}     # (row, hc) -> sigmoid activations
            u16 = {}     # (row, hc)
            g16 = {}     # row
            z0t = {}     # (row, hc) -> z0 scan output (kept for last column)

            def emit_x_dma(c):
                for row in range(B_core):
                    for kc in range(KC):
                        xt = xpool.tile([128, C], f16, tag=f"x_{row}_{kc}")
                        nc.sync.dma_start(xt[:], xc[c, row, kc])
                        x16[row, kc] = xt

            def emit_g(row):
                gp = ps_proj.tile([R_, C], f32, tag="proj")
                for kc in range(KC):
                    nc.tensor.matmul(
                        gp[:], wg16[:, kc, :], x16[row, kc][:],
                        start=(kc == 0), stop=(kc == KC - 1))
                gt = spool.tile([R_, C], f16, tag=f"g_{row}")
                nc.scalar.activation(gt[:], gp[:], AF.Identity, bias=bg_t[:])
                g16[row] = gt

            def emit_a(row, hc):
                hs = slice(hc * 128, (hc + 1) * 128)
                ap = ps_proj.tile([128, C], f32, tag="proj")
                for kc in range(KC):
                    nc.tensor.matmul(
                        ap[:], wa16[:, kc, hs], x16[row, kc][:],
                        start=(kc == 0), stop=(kc == KC - 1))
                at = apool.tile([128, C], f16, tag=f"a_{row}_{hc}")
                nc.scalar.activation(
                    at[:], ap[:], AF.Sigmoid, bias=ba_t[:, hc:hc + 1])
                a16[row, hc] = at

            def emit_u(row, hc):
                hs = slice(hc * 128, (hc + 1) * 128)
                up = ps_proj.tile([128, C], f32, tag="proj")
                for kc in range(KC):
                    nc.tensor.matmul(
                        up[:], wu16[:, kc, hs], x16[row, kc][:],
                        start=(kc == 0), stop=(kc == KC - 1))
                ut = upool.tile([128, C], f16, tag=f"u_{row}_{hc}")
                nc.scalar.activation(
                    ut[:], up[:], AF.Identity, bias=bu_t[:, hc:hc + 1])
                u16[row, hc] = ut

            # ---------- pipelined chunk loop ----------
            # prologue: projections of chunk 0
            emit_x_dma(0)
            for row in range(B_core):
                emit_g(row)
            for hc in range(HC):
                for row in range(B_core):
                    emit_a(row, hc)
                for row in range(B_core):
                    emit_u(row, hc)

            for c in range(NCH):
                last = (c == NCH - 1)
                if not last:
                    emit_x_dma(c + 1)
                # P(c+1) emission queue, ordered so low-hc a/u tiles (the
                # first inputs the next chunk's scans need) finish first.
                pq = []
                if not last:
                    for hc in range(HC):
                        for row in range(B_core):
                            pq.append(lambda row=row, hc=hc: emit_a(row, hc))
                            pq.append(lambda row=row, hc=hc: emit_u(row, hc))
                    for row in range(B_core):
                        pq.append(lambda row=row: emit_g(row))

                # Phase A of S(c): z0 scans + V^T z0, round-robin with P(c+1)
                # projection groups so the PE always has dense ready work.
                p0ps = {}
                for hc in range(HC):
                    for row in range(B_core):
                        if hc == 0:
                            p0p = ps_p.tile([R_, C], f32, tag="p0")
                            p0ps[row] = p0p
                        col = row * HC + hc
                        z0 = zpool.tile([128, C], f16, tag=f"z_{row}_{hc}")
                        nc.vector.tensor_tensor_scan(
                            z0[:], a16[row, hc][:], u16[row, hc][:],
                            state16[:, col:col + 1], OP.mult, OP.add)
                        z0t[row, hc] = z0
                        nc.tensor.matmul(
                            p0ps[row][:], v16[:, hc, :], z0[:],
                            start=(hc == 0), stop=(hc == HC - 1))
                    for _ in range(3 if hc < HC - 1 else len(pq)):
                        if pq:
                            pq.pop(0)()

                # q build
                qts = {}
                for row in range(B_core):
                    qt = spool.tile([R_, C], f16, tag=f"q_{row}")
                    nc.vector.tensor_tensor(
                        qt[:, 1:C], g16[row][:, 1:C], p0ps[row][:, 0:C - 1],
                        OP.mult)
                    nc.vector.tensor_tensor(
                        qt[:, 0:1], g16[row][:, 0:1], p_first[:, row:row + 1],
                        OP.mult)
                    qts[row] = qt

                # Phase B of S(c): Uq + correction scans + state update +
                # p_first refresh (+ output DMA on the last chunk).
                for row in range(B_core):
                    for hc in range(HC):
                        hs = slice(hc * 128, (hc + 1) * 128)
                        col = row * HC + hc
                        uqp = ps_uq.tile([128, C], f32, tag="uq")
                        nc.tensor.matmul(
                            uqp[:], u16T[:, hs], qts[row][:],
                            start=True, stop=True)
                        cl = clpool.tile([128, C], f16, tag="cl")
                        nc.vector.tensor_tensor_scan(
                            cl[:], a16[row, hc][:], uqp[:], 0.0,
                            OP.mult, OP.add)
                        nc.vector.tensor_tensor(
                            state16[:, col:col + 1], z0t[row, hc][:, C - 1:C],
                            cl[:, C - 1:C], OP.add)
                    if not last:
                        pfp = ps_tiny.tile([R_, 1], f32, tag="pf")
                        for hc in range(HC):
                            col = row * HC + hc
                            nc.tensor.matmul(
                                pfp[:], v16[:, hc, :],
                                state16[:, col:col + 1],
                                start=(hc == 0), stop=(hc == HC - 1))
                        nc.vector.tensor_copy(
                            p_first[:, row:row + 1], pfp[:])
                    else:
                        rs = slice(row * HC, (row + 1) * HC)
                        nc.sync.dma_start(
                            out_d[row].rearrange("(hc p) -> p hc", p=128),
                            state16[:, rs])
    nc.finalize()
    return nc


def make_in_maps(x, Wa, ba, Wg, bg, Wu, bu, u, v, n_cores=N_CORES, C=CHUNK):
    """Shard + lay out host-side (layout transforms + fp16 casts)."""
    B_, S_, D_ = x.shape
    H_, R_ = u.shape
    KC, HC, NCH = D_ // 128, H_ // 128, S_ // C
    b_core = B_ // n_cores
    waT = np.ascontiguousarray(Wa.T).reshape(KC, 128, H_).astype(np.float16)
    wuT = np.ascontiguousarray(Wu.T).reshape(KC, 128, H_).astype(np.float16)
    wgT = np.ascontiguousarray(Wg.T).reshape(KC, 128, R_).astype(np.float16)
    uT = np.ascontiguousarray(u.T).astype(np.float16)
    vh = np.ascontiguousarray(
        v.reshape(HC, 128, R_).transpose(1, 0, 2)).astype(np.float16)
    ba_h = np.ascontiguousarray(ba.reshape(HC, 128).T).astype(np.float32)
    bu_h = np.ascontiguousarray(bu.reshape(HC, 128).T).astype(np.float32)
    bg_h = np.ascontiguousarray(bg.reshape(R_, 1)).astype(np.float32)
    in_maps = []
    for core in range(n_cores):
        rows = slice(core * b_core, (core + 1) * b_core)
        # [b, S, D] -> [NCH, b, KC, 128, C], fully contiguous per tile
        xcore = x[rows].astype(np.float16)
        xck = xcore.reshape(b_core, NCH, C, KC, 128)
        xc = np.ascontiguousarray(xck.transpose(1, 0, 3, 4, 2))
        in_maps.append({
            "xc": xc, "waT": waT, "wuT": wuT, "wgT": wgT, "uT": uT,
            "v": vh, "ba": ba_h, "bu": bu_h, "bg": bg_h,
        })
    return in_maps


def kernel(x, Wa, ba, Wg, bg, Wu, bu, u, v):
    x = np.asarray(x, dtype=np.float32)
    in_maps = make_in_maps(
        x, np.asarray(Wa), np.asarray(ba), np.asarray(Wg), np.asarray(bg),
        np.asarray(Wu), np.asarray(bu), np.asarray(u), np.asarray(v))
    nc = build_kernel()
    res = run_bass_kernel_spmd(nc, in_maps, core_ids=list(range(N_CORES)))
    return np.concatenate(
        [res.results[i]["out"].astype(np.float32) for i in range(N_CORES)],
        axis=0)


if __name__ == "__main__":
    import reference  # only when run manually next to reference.py

    inputs = {k: np.asarray(v) for k, v in reference.setup_inputs().items()}
    got = kernel(**inputs)
    exp = np.asarray(reference.reference(**inputs))
    print("relmax:", np.abs(got - exp).max() / np.abs(exp).max())


# revision 7
# speedup vs baseline: 1.1222x; 1.1222x over previous
"""Trainium2 Bass kernel for nn_AffineExpert (diag + rank-R linear recurrence).

Math: s_{t+1} = a_t*s_t + u_t + U (g_t * (V^T s_t)),  s_0 = 0, output s_S.
  a = sigmoid(x@Wa^T + ba), g = x@Wg^T + bg, u = x@Wu^T + bu.

Strategy per core (data-parallel over batch, 2 rows/core on 8 cores):

  * Heavy projections (a, u, g) are fp16 PE matmuls with fp32 PSUM
    accumulation, N=512 time-tiles; inputs staged fp16 on the host.
  * The recurrence is linear in the rank-R channel q_t = g_t*(V^T s_t).
    Per 512-step chunk: z0 = diag-decay scan of (a, u) from the carried
    state (DVE tensor_tensor_scan), p0 = V^T z0, q = g * shift(p0), one
    more scan of (a, U q) is the low-rank correction (single fixed-point
    pass; total error ~1.5e-3 vs the 2e-2 gate).
  * Deep software pipeline in "eras": era(c) executes on the PE
    [Uq(c), V^T(c+1)] plus ALL projection matmuls of chunk c+2 as
    filler, and on the DVE [cl(c,hc) -> state(c,hc) -> z0(c+1,hc)]
    per hc-step.  Every data gate is produced a full era ahead, so the
    PE runs dense (HAM stays warm) while the DVE scan chain floats
    with ~20us of slack per era.
"""
import numpy as np

import concourse.bass as bass
import concourse.mybir as mybir
import concourse.tile as tile
from concourse import bacc
from concourse.bass_utils import run_bass_kernel_spmd
from concourse.tile import add_dep_helper

f32 = mybir.dt.float32
f16 = mybir.dt.float16
AF = mybir.ActivationFunctionType
OP = mybir.AluOpType

B, S, D, H, R = 16, 2048, 1024, 1024, 16
N_CORES = 8
B_CORE = B // N_CORES
CHUNK = 512


def build_kernel(B_core=B_CORE, S_=S, D_=D, H_=H, R_=R, C=CHUNK):
    KC, HC, NCH = D_ // 128, H_ // 128, S_ // C
    nc = bacc.Bacc("TRN2")

    xc = nc.dram_tensor("xc", [NCH, B_core, KC, 128, C], f16, kind="ExternalInput")
    waT = nc.dram_tensor("waT", [KC, 128, H_], f16, kind="ExternalInput")
    wuT = nc.dram_tensor("wuT", [KC, 128, H_], f16, kind="ExternalInput")
    wgT = nc.dram_tensor("wgT", [KC, 128, R_], f16, kind="ExternalInput")
    uT_d = nc.dram_tensor("uT", [R_, H_], f16, kind="ExternalInput")
    v_d = nc.dram_tensor("v", [128, HC, R_], f16, kind="ExternalInput")
    ba_d = nc.dram_tensor("ba", [128, HC], f32, kind="ExternalInput")
    bu_d = nc.dram_tensor("bu", [128, HC], f32, kind="ExternalInput")
    bg_d = nc.dram_tensor("bg", [R_, 1], f32, kind="ExternalInput")
    out_d = nc.dram_tensor("out", [B_core, H_], f16, kind="ExternalOutput")

    with tile.TileContext(nc) as tc:
        with tc.tile_pool(name="persist", bufs=1) as persist, \
             tc.tile_pool(name="xpool", bufs=2) as xpool, \
             tc.tile_pool(name="apool", bufs=3) as apool, \
             tc.tile_pool(name="upool", bufs=2) as upool, \
             tc.tile_pool(name="zpool", bufs=2) as zpool, \
             tc.tile_pool(name="spool", bufs=3) as spool, \
             tc.tile_pool(name="clpool", bufs=4) as clpool, \
             tc.tile_pool(name="ps_proj", bufs=3, space="PSUM") as ps_proj, \
             tc.tile_pool(name="ps_uq", bufs=2, space="PSUM") as ps_uq, \
             tc.tile_pool(name="ps_p", bufs=2, space="PSUM") as ps_p, \
             tc.tile_pool(name="ps_tiny", bufs=1, space="PSUM") as ps_tiny:

            # ---------- persistent staging ----------
            wa16 = persist.tile([128, KC, H_], f16)
            wu16 = persist.tile([128, KC, H_], f16)
            wg16 = persist.tile([128, KC, R_], f16)
            v16 = persist.tile([128, HC, R_], f16)
            u16T = persist.tile([R_, H_], f16)
            ba_t = persist.tile([128, HC], f32)
            bu_t = persist.tile([128, HC], f32)
            bg_t = persist.tile([R_, 1], f32)
            state16 = persist.tile([128, B_core * HC], f16)
            p_first = persist.tile([R_, B_core], f32)

            # staging spread across sync/scalar/gpsimd DGE queues
            for kc in range(KC):
                nc.scalar.dma_start(wg16[:, kc, :], wgT[kc])
            for kc in range(KC):
                nc.gpsimd.dma_start(wa16[:, kc, :], waT[kc])
            for kc in range(KC):
                nc.scalar.dma_start(wu16[:, kc, :], wuT[kc])
            nc.gpsimd.dma_start(v16[:], v_d[:, :, :])
            nc.gpsimd.dma_start(u16T[:], uT_d[:, :])
            nc.scalar.dma_start(ba_t[:], ba_d[:, :])
            nc.gpsimd.dma_start(bu_t[:], bu_d[:, :])
            nc.scalar.dma_start(bg_t[:], bg_d[:, :])

            nc.vector.memset(state16[:], 0.0)
            nc.vector.memset(p_first[:], 0.0)

            # ---------- emission helpers ----------
            x16 = {}     # (chunk, row, kc)
            a16 = {}     # (chunk, row, hc)
            u16 = {}     # (chunk, row, hc)
            g16 = {}     # (chunk, row)
            z0t = {}     # (chunk, row, hc)
            q16 = {}     # (chunk, row)
            p0ps = {}    # (chunk, row)
            last_s_mm = [None]   # most recent S-chain matmul (ordering anchor)

            def emit_x_dma(c):
                for row in range(B_core):
                    for kc in range(KC):
                        xt = xpool.tile([128, C], f16, tag=f"x_{row}_{kc}")
                        nc.sync.dma_start(xt[:], xc[c, row, kc])
                        x16[c, row, kc] = xt

            def order_after_s(mm):
                if last_s_mm[0] is not None:
                    add_dep_helper(
                        mm.ins, last_s_mm[0].ins, sync=False,
                        reason="projection filler after S-chain mm")

            def emit_g(c, row):
                gp = ps_proj.tile([R_, C], f32, tag="proj")
                for kc in range(KC):
                    mm = nc.tensor.matmul(
                        gp[:], wg16[:, kc, :], x16[c, row, kc][:],
                        start=(kc == 0), stop=(kc == KC - 1))
                    if kc == 0:
                        order_after_s(mm)
                gt = spool.tile([R_, C], f16, tag=f"g_{row}")
                nc.scalar.activation(gt[:], gp[:], AF.Identity, bias=bg_t[:])
                g16[c, row] = gt

            def emit_a(c, row, hc):
                hs = slice(hc * 128, (hc + 1) * 128)
                ap = ps_proj.tile([128, C], f32, tag="proj")
                for kc in range(KC):
                    mm = nc.tensor.matmul(
                        ap[:], wa16[:, kc, hs], x16[c, row, kc][:],
                        start=(kc == 0), stop=(kc == KC - 1))
                    if kc == 0:
                        order_after_s(mm)
                at = apool.tile([128, C], f16, tag=f"a_{row}_{hc}")
                nc.scalar.activation(
                    at[:], ap[:], AF.Sigmoid, bias=ba_t[:, hc:hc + 1])
                a16[c, row, hc] = at

            def emit_u(c, row, hc):
                hs = slice(hc * 128, (hc + 1) * 128)
                up = ps_proj.tile([128, C], f32, tag="proj")
                for kc in range(KC):
                    mm = nc.tensor.matmul(
                        up[:], wu16[:, kc, hs], x16[c, row, kc][:],
                        start=(kc == 0), stop=(kc == KC - 1))
                    if kc == 0:
                        order_after_s(mm)
                ut = upool.tile([128, C], f16, tag=f"u_{row}_{hc}")
                nc.scalar.activation(
                    ut[:], up[:], AF.Identity, bias=bu_t[:, hc:hc + 1])
                u16[c, row, hc] = ut

            def build_pq(c):
                # hc-major so next era's early scans get their inputs first
                pq = []
                for hc in range(HC):
                    for row in range(B_core):
                        pq.append(lambda c=c, row=row, hc=hc: emit_a(c, row, hc))
                        pq.append(lambda c=c, row=row, hc=hc: emit_u(c, row, hc))
                for row in range(B_core):
                    pq.append(lambda c=c, row=row: emit_g(c, row))
                return pq

            def emit_z0_vt(c, row, hc):
                # z0 scan of chunk c + V^T accumulation (p0 of chunk c)
                col = row * HC + hc
                if hc == 0:
                    p0p = ps_p.tile([R_, C], f32, tag="p0")
                    p0ps[c, row] = p0p
                z0 = zpool.tile([128, C], f16, tag=f"z_{row}_{hc}")
                nc.vector.tensor_tensor_scan(
                    z0[:], a16[c, row, hc][:], u16[c, row, hc][:],
                    state16[:, col:col + 1], OP.mult, OP.add)
                z0t[c, row, hc] = z0
                mm = nc.tensor.matmul(
                    p0ps[c, row][:], v16[:, hc, :], z0[:],
                    start=(hc == 0), stop=(hc == HC - 1))
                last_s_mm[0] = mm

            def emit_q(c):
                for row in range(B_core):
                    qt = spool.tile([R_, C], f16, tag=f"q_{row}")
                    nc.vector.tensor_tensor(
                        qt[:, 1:C], g16[c, row][:, 1:C],
                        p0ps[c, row][:, 0:C - 1], OP.mult)
                    nc.vector.tensor_tensor(
                        qt[:, 0:1], g16[c, row][:, 0:1],
                        p_first[:, row:row + 1], OP.mult)
                    q16[c, row] = qt

            def emit_uq_cl(c, row, hc):
                # Uq matmul + correction scan + state update for chunk c
                hs = slice(hc * 128, (hc + 1) * 128)
                col = row * HC + hc
                uqp = ps_uq.tile([128, C], f32, tag="uq")
                mm = nc.tensor.matmul(
                    uqp[:], u16T[:, hs], q16[c, row][:], start=True, stop=True)
                last_s_mm[0] = mm
                cl = clpool.tile([128, C], f16, tag="cl")
                nc.vector.tensor_tensor_scan(
                    cl[:], a16[c, row, hc][:], uqp[:], 0.0, OP.mult, OP.add)
                nc.vector.tensor_tensor(
                    state16[:, col:col + 1], z0t[c, row, hc][:, C - 1:C],
                    cl[:, C - 1:C], OP.add)

            def emit_pfirst(c, row):
                pfp = ps_tiny.tile([R_, 1], f32, tag="pf")
                for hc in range(HC):
                    col = row * HC + hc
                    mm = nc.tensor.matmul(
                        pfp[:], v16[:, hc, :], state16[:, col:col + 1],
                        start=(hc == 0), stop=(hc == HC - 1))
                    last_s_mm[0] = mm
                nc.vector.tensor_copy(p_first[:, row:row + 1], pfp[:])

            # ---------- prologue ----------
            emit_x_dma(0)
            if NCH > 1:
                emit_x_dma(1)
            for row in range(B_core):
                emit_g(0, row)
            for hc in range(HC):
                for row in range(B_core):
                    emit_a(0, row, hc)
                for row in range(B_core):
                    emit_u(0, row, hc)

            # era A0: z0(0)+V^T(0), interposed with projections of chunk 1
            pq = build_pq(1) if NCH > 1 else []
            for hc in range(HC):
                for row in range(B_core):
                    emit_z0_vt(0, row, hc)
                for _ in range(2 if hc < HC - 1 else len(pq)):
                    if pq:
                        pq.pop(0)()
            emit_q(0)

            # ---------- steady eras ----------
            # era(c): Uq/cl/state of chunk c, z0/V^T of chunk c+1,
            # projections of chunk c+2 as PE filler.
            for c in range(NCH - 1):
                pq = []
                if c + 2 < NCH:
                    emit_x_dma(c + 2)
                    pq = build_pq(c + 2)
                for hc in range(HC):
                    for row in range(B_core):
                        emit_uq_cl(c, row, hc)
                        emit_z0_vt(c + 1, row, hc)
                    for _ in range(4 if hc < HC - 1 else len(pq)):
                        if pq:
                            pq.pop(0)()
                for row in range(B_core):
                    emit_pfirst(c, row)
                emit_q(c + 1)

            # ---------- final era: chunk NCH-1 correction + output ----------
            cL = NCH - 1
            for row in range(B_core):
                for hc in range(HC):
                    emit_uq_cl(cL, row, hc)
                rs = slice(row * HC, (row + 1) * HC)
                nc.sync.dma_start(
                    out_d[row].rearrange("(hc p) -> p hc", p=128),
                    state16[:, rs])
    nc.finalize()
    return nc


def make_in_maps(x, Wa, ba, Wg, bg, Wu, bu, u, v, n_cores=N_CORES, C=CHUNK):
    """Shard + lay out host-side (layout transforms + fp16 casts)."""
    B_, S_, D_ = x.shape
    H_, R_ = u.shape
    KC, HC, NCH = D_ // 128, H_ // 128, S_ // C
    b_core = B_ // n_cores
    waT = np.ascontiguousarray(Wa.T).reshape(KC, 128, H_).astype(np.float16)
    wuT = np.ascontiguousarray(Wu.T).reshape(KC, 128, H_).astype(np.float16)
    wgT = np.ascontiguousarray(Wg.T).reshape(KC, 128, R_).astype(np.float16)
    uT = np.ascontiguousarray(u.T).astype(np.float16)
    vh = np.ascontiguousarray(
        v.reshape(HC, 128, R_).transpose(1, 0, 2)).astype(np.float16)
    ba_h = np.ascontiguousarray(ba.reshape(HC, 128).T).astype(np.float32)
    bu_h = np.ascontiguousarray(bu.reshape(HC, 128).T).astype(np.float32)
    bg_h = np.ascontiguousarray(bg.reshape(R_, 1)).astype(np.float32)
    in_maps = []
    for core in range(n_cores):
        rows = slice(core * b_core, (core + 1) * b_core)
        xcore = x[rows].astype(np.float16)
        xck = xcore.reshape(b_core, NCH, C, KC, 128)
        xc = np.ascontiguousarray(xck.transpose(1, 0, 3, 4, 2))
        in_maps.append({
            "xc": xc, "waT": waT, "wuT": wuT, "wgT": wgT, "uT": uT,
            "v": vh, "ba": ba_h, "bu": bu_h, "bg": bg_h,
        })
    return in_maps


def kernel(x, Wa, ba, Wg, bg, Wu, bu, u, v):
    x = np.asarray(x, dtype=np.float32)
    in_maps = make_in_maps(
        x, np.asarray(Wa), np.asarray(ba), np.asarray(Wg), np.asarray(bg),
        np.asarray(Wu), np.asarray(bu), np.asarray(u), np.asarray(v))
    nc = build_kernel()
    res = run_bass_kernel_spmd(nc, in_maps, core_ids=list(range(N_CORES)))
    return np.concatenate(
        [res.results[i]["out"].astype(np.float32) for i in range(N_CORES)],
        axis=0)


if __name__ == "__main__":
    import reference  # only when run manually next to reference.py

    inputs = {k: np.asarray(v) for k, v in reference.setup_inputs().items()}
    got = kernel(**inputs)
    exp = np.asarray(reference.reference(**inputs))
    print("relmax:", np.abs(got - exp).max() / np.abs(exp).max())


# revision 11
# speedup vs baseline: 1.1225x; 1.0002x over previous
"""Trainium2 Bass kernel for nn_AffineExpert (diag + rank-R linear recurrence).

Math: s_{t+1} = a_t*s_t + u_t + U (g_t * (V^T s_t)),  s_0 = 0, output s_S.
  a = sigmoid(x@Wa^T + ba), g = x@Wg^T + bg, u = x@Wu^T + bu.

Strategy per core (data-parallel over batch, 2 rows/core on 8 cores):

  * Heavy projections (a, u, g) are fp16 PE matmuls with fp32 PSUM
    accumulation, N=512 time-tiles; inputs staged fp16 on the host.
  * The recurrence is linear in the rank-R channel q_t = g_t*(V^T s_t).
    Per 512-step chunk: z0 = diag-decay scan of (a, u) from the carried
    state (DVE tensor_tensor_scan), p0 = V^T z0, q = g * shift(p0), one
    more scan of (a, U q) is the low-rank correction (single fixed-point
    pass; total error ~1.5e-3 vs the 2e-2 gate).
  * Deep software pipeline in "eras": era(c) executes on the PE
    [Uq(c), V^T(c+1)] plus ALL projection matmuls of chunk c+2 as
    filler, and on the DVE [cl(c,hc) -> state(c,hc) -> z0(c+1,hc)]
    per hc-step.  Every data gate is produced a full era ahead, so the
    PE runs dense (HAM stays warm) while the DVE scan chain floats
    with ~20us of slack per era.
"""
import numpy as np

import concourse.bass as bass
import concourse.mybir as mybir
import concourse.tile as tile
from concourse import bacc
from concourse.bass_utils import run_bass_kernel_spmd
from concourse.tile import add_dep_helper

f32 = mybir.dt.float32
f16 = mybir.dt.float16
AF = mybir.ActivationFunctionType
OP = mybir.AluOpType

B, S, D, H, R = 16, 2048, 1024, 1024, 16
N_CORES = 8
B_CORE = B // N_CORES
CHUNK = 512


def build_kernel(B_core=B_CORE, S_=S, D_=D, H_=H, R_=R, C=CHUNK):
    KC, HC, NCH = D_ // 128, H_ // 128, S_ // C
    nc = bacc.Bacc("TRN2")

    xc = nc.dram_tensor("xc", [NCH, B_core, KC, 128, C], f16, kind="ExternalInput")
    waT = nc.dram_tensor("waT", [KC, 128, H_], f16, kind="ExternalInput")
    wuT = nc.dram_tensor("wuT", [KC, 128, H_], f16, kind="ExternalInput")
    wgT = nc.dram_tensor("wgT", [KC, 128, R_], f16, kind="ExternalInput")
    uT_d = nc.dram_tensor("uT", [R_, H_], f16, kind="ExternalInput")
    v_d = nc.dram_tensor("v", [128, HC, R_], f16, kind="ExternalInput")
    ba_d = nc.dram_tensor("ba", [128, HC], f32, kind="ExternalInput")
    bu_d = nc.dram_tensor("bu", [128, HC], f32, kind="ExternalInput")
    bg_d = nc.dram_tensor("bg", [R_, 1], f32, kind="ExternalInput")
    out_d = nc.dram_tensor("out", [B_core, H_], f16, kind="ExternalOutput")

    with tile.TileContext(nc) as tc:
        with tc.tile_pool(name="persist", bufs=1) as persist, \
             tc.tile_pool(name="xpool", bufs=2) as xpool, \
             tc.tile_pool(name="apool", bufs=3) as apool, \
             tc.tile_pool(name="upool", bufs=2) as upool, \
             tc.tile_pool(name="zpool", bufs=2) as zpool, \
             tc.tile_pool(name="spool", bufs=3) as spool, \
             tc.tile_pool(name="clpool", bufs=4) as clpool, \
             tc.tile_pool(name="ps_proj", bufs=3, space="PSUM") as ps_proj, \
             tc.tile_pool(name="ps_uq", bufs=2, space="PSUM") as ps_uq, \
             tc.tile_pool(name="ps_p", bufs=2, space="PSUM") as ps_p, \
             tc.tile_pool(name="ps_tiny", bufs=1, space="PSUM") as ps_tiny:

            # ---------- persistent staging ----------
            wa16 = persist.tile([128, KC, H_], f16)
            wu16 = persist.tile([128, KC, H_], f16)
            wg16 = persist.tile([128, KC, R_], f16)
            v16 = persist.tile([128, HC, R_], f16)
            u16T = persist.tile([R_, H_], f16)
            ba_t = persist.tile([128, HC], f32)
            bu_t = persist.tile([128, HC], f32)
            bg_t = persist.tile([R_, 1], f32)
            state16 = persist.tile([128, B_core * HC], f16)
            p_first = persist.tile([R_, B_core], f32)

            # staging spread across sync/scalar/gpsimd DGE queues
            for kc in range(KC):
                nc.scalar.dma_start(wg16[:, kc, :], wgT[kc])
            for kc in range(KC):
                nc.gpsimd.dma_start(wa16[:, kc, :], waT[kc])
            for kc in range(KC):
                nc.scalar.dma_start(wu16[:, kc, :], wuT[kc])
            nc.gpsimd.dma_start(v16[:], v_d[:, :, :])
            nc.gpsimd.dma_start(u16T[:], uT_d[:, :])
            nc.scalar.dma_start(ba_t[:], ba_d[:, :])
            nc.gpsimd.dma_start(bu_t[:], bu_d[:, :])
            nc.scalar.dma_start(bg_t[:], bg_d[:, :])

            nc.vector.memset(state16[:], 0.0)
            nc.vector.memset(p_first[:], 0.0)

            # ---------- emission helpers ----------
            x16 = {}     # (chunk, row, kc)
            a16 = {}     # (chunk, row, hc)
            u16 = {}     # (chunk, row, hc)
            g16 = {}     # (chunk, row)
            z0t = {}     # (chunk, row, hc)
            q16 = {}     # (chunk, row)
            p0ps = {}    # (chunk, row)
            s_mms = []           # recent S-chain matmuls (ordering anchors)
            S_LAG = 6            # filler may run ~6 S-MMs (3 sub-steps) ahead

            def emit_x_dma(c):
                for row in range(B_core):
                    for kc in range(KC):
                        xt = xpool.tile([128, C], f16, tag=f"x_{row}_{kc}")
                        nc.sync.dma_start(xt[:], xc[c, row, kc])
                        x16[c, row, kc] = xt

            def order_after_s(mm):
                # Bound the scheduler's projection runahead: filler may not
                # start before the S-chain matmul S_LAG slots back.  This
                # prevents flooding the PE queue with bulk work ahead of the
                # latency-critical chain, while leaving enough elasticity to
                # bridge the scan-latency bubbles at era boundaries.
                if len(s_mms) > S_LAG:
                    add_dep_helper(
                        mm.ins, s_mms[-S_LAG].ins, sync=False,
                        reason="projection filler after lagged S-chain mm")

            def emit_g(c, row):
                gp = ps_proj.tile([R_, C], f32, tag="proj")
                for kc in range(KC):
                    mm = nc.tensor.matmul(
                        gp[:], wg16[:, kc, :], x16[c, row, kc][:],
                        start=(kc == 0), stop=(kc == KC - 1))
                    if kc == 0:
                        order_after_s(mm)
                gt = spool.tile([R_, C], f16, tag=f"g_{row}")
                nc.scalar.activation(gt[:], gp[:], AF.Identity, bias=bg_t[:])
                g16[c, row] = gt

            def emit_a(c, row, hc):
                hs = slice(hc * 128, (hc + 1) * 128)
                ap = ps_proj.tile([128, C], f32, tag="proj")
                for kc in range(KC):
                    mm = nc.tensor.matmul(
                        ap[:], wa16[:, kc, hs], x16[c, row, kc][:],
                        start=(kc == 0), stop=(kc == KC - 1))
                    if kc == 0:
                        order_after_s(mm)
                at = apool.tile([128, C], f16, tag=f"a_{row}_{hc}")
                nc.scalar.activation(
                    at[:], ap[:], AF.Sigmoid, bias=ba_t[:, hc:hc + 1])
                a16[c, row, hc] = at

            def emit_u(c, row, hc):
                hs = slice(hc * 128, (hc + 1) * 128)
                up = ps_proj.tile([128, C], f32, tag="proj")
                for kc in range(KC):
                    mm = nc.tensor.matmul(
                        up[:], wu16[:, kc, hs], x16[c, row, kc][:],
                        start=(kc == 0), stop=(kc == KC - 1))
                    if kc == 0:
                        order_after_s(mm)
                ut = upool.tile([128, C], f16, tag=f"u_{row}_{hc}")
                nc.scalar.activation(
                    ut[:], up[:], AF.Identity, bias=bu_t[:, hc:hc + 1])
                u16[c, row, hc] = ut

            def build_pq(c):
                # hc-major so next era's early scans get their inputs first
                pq = []
                for hc in range(HC):
                    for row in range(B_core):
                        pq.append(lambda c=c, row=row, hc=hc: emit_a(c, row, hc))
                        pq.append(lambda c=c, row=row, hc=hc: emit_u(c, row, hc))
                for row in range(B_core):
                    pq.append(lambda c=c, row=row: emit_g(c, row))
                return pq

            def emit_z0_vt(c, row, hc):
                # z0 scan of chunk c + V^T accumulation (p0 of chunk c)
                col = row * HC + hc
                if hc == 0:
                    p0p = ps_p.tile([R_, C], f32, tag="p0")
                    p0ps[c, row] = p0p
                z0 = zpool.tile([128, C], f16, tag=f"z_{row}_{hc}")
                nc.vector.tensor_tensor_scan(
                    z0[:], a16[c, row, hc][:], u16[c, row, hc][:],
                    state16[:, col:col + 1], OP.mult, OP.add)
                z0t[c, row, hc] = z0
                mm = nc.tensor.matmul(
                    p0ps[c, row][:], v16[:, hc, :], z0[:],
                    start=(hc == 0), stop=(hc == HC - 1))
                s_mms.append(mm)

            def emit_q(c):
                for row in range(B_core):
                    qt = spool.tile([R_, C], f16, tag=f"q_{row}")
                    nc.vector.tensor_tensor(
                        qt[:, 1:C], g16[c, row][:, 1:C],
                        p0ps[c, row][:, 0:C - 1], OP.mult)
                    nc.vector.tensor_tensor(
                        qt[:, 0:1], g16[c, row][:, 0:1],
                        p_first[:, row:row + 1], OP.mult)
                    q16[c, row] = qt

            def emit_uq_cl(c, row, hc):
                # Uq matmul + correction scan + state update for chunk c
                hs = slice(hc * 128, (hc + 1) * 128)
                col = row * HC + hc
                uqp = ps_uq.tile([128, C], f32, tag="uq")
                mm = nc.tensor.matmul(
                    uqp[:], u16T[:, hs], q16[c, row][:], start=True, stop=True)
                s_mms.append(mm)
                cl = clpool.tile([128, C], f16, tag="cl")
                nc.vector.tensor_tensor_scan(
                    cl[:], a16[c, row, hc][:], uqp[:], 0.0, OP.mult, OP.add)
                nc.vector.tensor_tensor(
                    state16[:, col:col + 1], z0t[c, row, hc][:, C - 1:C],
                    cl[:, C - 1:C], OP.add)

            def emit_pfirst(c, row):
                pfp = ps_tiny.tile([R_, 1], f32, tag="pf")
                for hc in range(HC):
                    col = row * HC + hc
                    mm = nc.tensor.matmul(
                        pfp[:], v16[:, hc, :], state16[:, col:col + 1],
                        start=(hc == 0), stop=(hc == HC - 1))
                    s_mms.append(mm)
                nc.vector.tensor_copy(p_first[:, row:row + 1], pfp[:])

            # ---------- prologue ----------
            emit_x_dma(0)
            if NCH > 1:
                emit_x_dma(1)
            for row in range(B_core):
                emit_g(0, row)
            for hc in range(HC):
                for row in range(B_core):
                    emit_a(0, row, hc)
                for row in range(B_core):
                    emit_u(0, row, hc)

            # era A0: z0(0)+V^T(0), interposed with projections of chunk 1
            pq = build_pq(1) if NCH > 1 else []
            for hc in range(HC):
                for row in range(B_core):
                    emit_z0_vt(0, row, hc)
                    for _ in range(1):
                        if pq:
                            pq.pop(0)()
            emit_q(0)
            while pq:
                pq.pop(0)()

            # ---------- steady eras ----------
            # era(c): Uq/cl/state of chunk c, z0/V^T of chunk c+1,
            # projections of chunk c+2 as PE filler.
            for c in range(NCH - 1):
                pq = []
                if c + 2 < NCH:
                    emit_x_dma(c + 2)
                    pq = build_pq(c + 2)
                for hc in range(HC):
                    for row in range(B_core):
                        emit_uq_cl(c, row, hc)
                        emit_z0_vt(c + 1, row, hc)
                        for _ in range(2):
                            if pq:
                                pq.pop(0)()
                for row in range(B_core):
                    emit_pfirst(c, row)
                emit_q(c + 1)
                while pq:
                    pq.pop(0)()

            # ---------- final era: chunk NCH-1 correction + output ----------
            cL = NCH - 1
            for row in range(B_core):
                for hc in range(HC):
                    emit_uq_cl(cL, row, hc)
                rs = slice(row * HC, (row + 1) * HC)
                nc.sync.dma_start(
                    out_d[row].rearrange("(hc p) -> p hc", p=128),
                    state16[:, rs])
    nc.finalize()
    return nc


def make_in_maps(x, Wa, ba, Wg, bg, Wu, bu, u, v, n_cores=N_CORES, C=CHUNK):
    """Shard + lay out host-side (layout transforms + fp16 casts)."""
    B_, S_, D_ = x.shape
    H_, R_ = u.shape
    KC, HC, NCH = D_ // 128, H_ // 128, S_ // C
    b_core = B_ // n_cores
    waT = np.ascontiguousarray(Wa.T).reshape(KC, 128, H_).astype(np.float16)
    wuT = np.ascontiguousarray(Wu.T).reshape(KC, 128, H_).astype(np.float16)
    wgT = np.ascontiguousarray(Wg.T).reshape(KC, 128, R_).astype(np.float16)
    uT = np.ascontiguousarray(u.T).astype(np.float16)
    vh = np.ascontiguousarray(
        v.reshape(HC, 128, R_).transpose(1, 0, 2)).astype(np.float16)
    ba_h = np.ascontiguousarray(ba.reshape(HC, 128).T).astype(np.float32)
    bu_h = np.ascontiguousarray(bu.reshape(HC, 128).T).astype(np.float32)
    bg_h = np.ascontiguousarray(bg.reshape(R_, 1)).astype(np.float32)
    in_maps = []
    for core in range(n_cores):
        rows = slice(core * b_core, (core + 1) * b_core)
        xcore = x[rows].astype(np.float16)
        xck = xcore.reshape(b_core, NCH, C, KC, 128)
        xc = np.ascontiguousarray(xck.transpose(1, 0, 3, 4, 2))
        in_maps.append({
            "xc": xc, "waT": waT, "wuT": wuT, "wgT": wgT, "uT": uT,
            "v": vh, "ba": ba_h, "bu": bu_h, "bg": bg_h,
        })
    return in_maps


def kernel(x, Wa, ba, Wg, bg, Wu, bu, u, v):
    x = np.asarray(x, dtype=np.float32)
    in_maps = make_in_maps(
        x, np.asarray(Wa), np.asarray(ba), np.asarray(Wg), np.asarray(bg),
        np.asarray(Wu), np.asarray(bu), np.asarray(u), np.asarray(v))
    nc = build_kernel()
    res = run_bass_kernel_spmd(nc, in_maps, core_ids=list(range(N_CORES)))
    return np.concatenate(
        [res.results[i]["out"].astype(np.float32) for i in range(N_CORES)],
        axis=0)


if __name__ == "__main__":
    import reference  # only when run manually next to reference.py

    inputs = {k: np.asarray(v) for k, v in reference.setup_inputs().items()}
    got = kernel(**inputs)
    exp = np.asarray(reference.reference(**inputs))
    print("relmax:", np.abs(got - exp).max() / np.abs(exp).max())


# revision 13
# speedup vs baseline: 1.2402x; 1.1049x over previous
"""Trainium2 Bass kernel for nn_AffineExpert (diag + rank-R linear recurrence).

Math: s_{t+1} = a_t*s_t + u_t + U (g_t * (V^T s_t)),  s_0 = 0, output s_S.
  a = sigmoid(x@Wa^T + ba), g = x@Wg^T + bg, u = x@Wu^T + bu.

Strategy per core (data-parallel over batch, 2 rows/core on 8 cores):

  * Heavy projections (a, u, g) are fp16 PE matmuls with fp32 PSUM
    accumulation, N=512 time-tiles; inputs staged fp16 on the host.
  * The recurrence is linear in the rank-R channel q_t = g_t*(V^T s_t).
    Per 512-step chunk: z0 = diag-decay scan of (a, u) from the carried
    state (DVE tensor_tensor_scan), p0 = V^T z0, q = g * shift(p0), one
    more scan of (a, U q) is the low-rank correction (single fixed-point
    pass; total error ~1.5e-3 vs the 2e-2 gate).
  * Deep software pipeline in "eras": era(c) executes on the PE
    [Uq(c), V^T(c+1)] plus ALL projection matmuls of chunk c+2 as
    filler, and on the DVE [cl(c,hc) -> state(c,hc) -> z0(c+1,hc)]
    per hc-step.  Every data gate is produced a full era ahead, so the
    PE runs dense (HAM stays warm) while the DVE scan chain floats
    with ~20us of slack per era.
"""
import numpy as np

import concourse.bass as bass
import concourse.mybir as mybir
import concourse.tile as tile
from concourse import bacc
from concourse.bass_utils import run_bass_kernel_spmd
from concourse.tile import add_dep_helper

f32 = mybir.dt.float32
f16 = mybir.dt.float16
AF = mybir.ActivationFunctionType
OP = mybir.AluOpType

B, S, D, H, R = 16, 2048, 1024, 1024, 16
N_CORES = 8
B_CORE = B // N_CORES
CHUNK = 512


def build_kernel(B_core=B_CORE, S_=S, D_=D, H_=H, R_=R, C=CHUNK):
    KC, HC, NCH = D_ // 128, H_ // 128, S_ // C
    nc = bacc.Bacc("TRN2")

    xc = nc.dram_tensor("xc", [NCH, B_core, KC, 128, C], f16, kind="ExternalInput")
    waT = nc.dram_tensor("waT", [KC, 128, H_], f16, kind="ExternalInput")
    wuT = nc.dram_tensor("wuT", [KC, 128, H_], f16, kind="ExternalInput")
    wgT = nc.dram_tensor("wgT", [KC, 128, R_], f16, kind="ExternalInput")
    uT_d = nc.dram_tensor("uT", [R_, H_], f16, kind="ExternalInput")
    v_d = nc.dram_tensor("v", [128, HC, R_], f16, kind="ExternalInput")
    ba_d = nc.dram_tensor("ba", [128, HC], f32, kind="ExternalInput")
    bu_d = nc.dram_tensor("bu", [128, HC], f32, kind="ExternalInput")
    bg_d = nc.dram_tensor("bg", [R_, 1], f32, kind="ExternalInput")
    out_d = nc.dram_tensor("out", [B_core, H_], f16, kind="ExternalOutput")

    with tile.TileContext(nc) as tc:
        with tc.tile_pool(name="persist", bufs=1) as persist, \
             tc.tile_pool(name="xpool", bufs=2) as xpool, \
             tc.tile_pool(name="apool", bufs=3) as apool, \
             tc.tile_pool(name="upool", bufs=2) as upool, \
             tc.tile_pool(name="zpool", bufs=2) as zpool, \
             tc.tile_pool(name="spool", bufs=3) as spool, \
             tc.tile_pool(name="clpool", bufs=4) as clpool, \
             tc.tile_pool(name="ps_proj", bufs=3, space="PSUM") as ps_proj, \
             tc.tile_pool(name="ps_uq", bufs=2, space="PSUM") as ps_uq, \
             tc.tile_pool(name="ps_p", bufs=2, space="PSUM") as ps_p, \
             tc.tile_pool(name="ps_tiny", bufs=1, space="PSUM") as ps_tiny:

            # ---------- persistent staging ----------
            wa16 = persist.tile([128, KC, H_], f16)
            wu16 = persist.tile([128, KC, H_], f16)
            wg16 = persist.tile([128, KC, R_], f16)
            v16 = persist.tile([128, HC, R_], f16)
            u16T = persist.tile([R_, H_], f16)
            ba_t = persist.tile([128, HC], f32)
            bu_t = persist.tile([128, HC], f32)
            bg_t = persist.tile([R_, 1], f32)
            state16 = persist.tile([128, B_core * HC], f16)
            p_first = persist.tile([R_, B_core], f32)

            # staging spread across sync/scalar/gpsimd DGE queues
            for kc in range(KC):
                nc.scalar.dma_start(wg16[:, kc, :], wgT[kc])
            for kc in range(KC):
                nc.gpsimd.dma_start(wa16[:, kc, :], waT[kc])
            for kc in range(KC):
                nc.scalar.dma_start(wu16[:, kc, :], wuT[kc])
            nc.gpsimd.dma_start(v16[:], v_d[:, :, :])
            nc.gpsimd.dma_start(u16T[:], uT_d[:, :])
            nc.scalar.dma_start(ba_t[:], ba_d[:, :])
            nc.gpsimd.dma_start(bu_t[:], bu_d[:, :])
            nc.scalar.dma_start(bg_t[:], bg_d[:, :])

            nc.vector.memset(state16[:], 0.0)
            nc.vector.memset(p_first[:], 0.0)

            # ---------- emission helpers ----------
            x16 = {}     # (chunk, row, kc)
            a16 = {}     # (chunk, row, hc)
            u16 = {}     # (chunk, row, hc)
            g16 = {}     # (chunk, row)
            z0t = {}     # (chunk, row, hc)
            q16 = {}     # (chunk, row)
            p0ps = {}    # (chunk, row)
            s_mms = []           # recent S-chain matmuls (ordering anchors)
            S_LAG = 6            # filler may run ~6 S-MMs (3 sub-steps) ahead

            def emit_x_dma(c):
                for row in range(B_core):
                    for kc in range(KC):
                        xt = xpool.tile([128, C], f16, tag=f"x_{row}_{kc}")
                        nc.sync.dma_start(xt[:], xc[c, row, kc])
                        x16[c, row, kc] = xt

            def order_after_s(mm):
                # Bound the scheduler's projection runahead: filler may not
                # start before the S-chain matmul S_LAG slots back.  This
                # prevents flooding the PE queue with bulk work ahead of the
                # latency-critical chain, while leaving enough elasticity to
                # bridge the scan-latency bubbles at era boundaries.
                if len(s_mms) > S_LAG:
                    add_dep_helper(
                        mm.ins, s_mms[-S_LAG].ins, sync=False,
                        reason="projection filler after lagged S-chain mm")

            def emit_g(c, row):
                gp = ps_proj.tile([R_, C], f32, tag="proj")
                for kc in range(KC):
                    mm = nc.tensor.matmul(
                        gp[:], wg16[:, kc, :], x16[c, row, kc][:],
                        start=(kc == 0), stop=(kc == KC - 1))
                    if kc == 0:
                        order_after_s(mm)
                gt = spool.tile([R_, C], f16, tag=f"g_{row}")
                nc.scalar.activation(gt[:], gp[:], AF.Identity, bias=bg_t[:])
                g16[c, row] = gt
                return mm

            def emit_a(c, row, hc):
                hs = slice(hc * 128, (hc + 1) * 128)
                ap = ps_proj.tile([128, C], f32, tag="proj")
                for kc in range(KC):
                    mm = nc.tensor.matmul(
                        ap[:], wa16[:, kc, hs], x16[c, row, kc][:],
                        start=(kc == 0), stop=(kc == KC - 1))
                    if kc == 0:
                        order_after_s(mm)
                at = apool.tile([128, C], f16, tag=f"a_{row}_{hc}")
                nc.scalar.activation(
                    at[:], ap[:], AF.Sigmoid, bias=ba_t[:, hc:hc + 1])
                a16[c, row, hc] = at
                return mm

            def emit_u(c, row, hc):
                hs = slice(hc * 128, (hc + 1) * 128)
                up = ps_proj.tile([128, C], f32, tag="proj")
                for kc in range(KC):
                    mm = nc.tensor.matmul(
                        up[:], wu16[:, kc, hs], x16[c, row, kc][:],
                        start=(kc == 0), stop=(kc == KC - 1))
                    if kc == 0:
                        order_after_s(mm)
                ut = upool.tile([128, C], f16, tag=f"u_{row}_{hc}")
                nc.scalar.activation(
                    ut[:], up[:], AF.Identity, bias=bu_t[:, hc:hc + 1])
                u16[c, row, hc] = ut
                return mm

            def build_pq(c):
                # hc-major so next era's early scans get their inputs first
                pq = []
                for hc in range(HC):
                    for row in range(B_core):
                        pq.append(lambda c=c, row=row, hc=hc: emit_a(c, row, hc))
                        pq.append(lambda c=c, row=row, hc=hc: emit_u(c, row, hc))
                for row in range(B_core):
                    pq.append(lambda c=c, row=row: emit_g(c, row))
                return pq

            def emit_z0_vt(c, row, hc, after=None):
                # z0 scan of chunk c + V^T accumulation (p0 of chunk c)
                col = row * HC + hc
                if hc == 0:
                    p0p = ps_p.tile([R_, C], f32, tag="p0")
                    p0ps[c, row] = p0p
                z0 = zpool.tile([128, C], f16, tag=f"z_{row}_{hc}")
                nc.vector.tensor_tensor_scan(
                    z0[:], a16[c, row, hc][:], u16[c, row, hc][:],
                    state16[:, col:col + 1], OP.mult, OP.add)
                z0t[c, row, hc] = z0
                mm = nc.tensor.matmul(
                    p0ps[c, row][:], v16[:, hc, :], z0[:],
                    start=(hc == 0), stop=(hc == HC - 1))
                if after is not None:
                    # The V^T matmul waits on its z0 scan (~2.6us of DVE
                    # work); force the sub-step's filler groups AHEAD of it
                    # in the in-order PE queue so they hide that latency.
                    add_dep_helper(
                        mm.ins, after.ins, sync=False,
                        reason="latency-bound V^T after sub-step fillers")
                s_mms.append(mm)

            def emit_q(c):
                for row in range(B_core):
                    qt = spool.tile([R_, C], f16, tag=f"q_{row}")
                    nc.vector.tensor_tensor(
                        qt[:, 1:C], g16[c, row][:, 1:C],
                        p0ps[c, row][:, 0:C - 1], OP.mult)
                    nc.vector.tensor_tensor(
                        qt[:, 0:1], g16[c, row][:, 0:1],
                        p_first[:, row:row + 1], OP.mult)
                    q16[c, row] = qt

            def emit_uq_cl(c, row, hc):
                # Uq matmul + correction scan + state update for chunk c
                hs = slice(hc * 128, (hc + 1) * 128)
                col = row * HC + hc
                uqp = ps_uq.tile([128, C], f32, tag="uq")
                mm = nc.tensor.matmul(
                    uqp[:], u16T[:, hs], q16[c, row][:], start=True, stop=True)
                s_mms.append(mm)
                cl = clpool.tile([128, C], f16, tag="cl")
                nc.vector.tensor_tensor_scan(
                    cl[:], a16[c, row, hc][:], uqp[:], 0.0, OP.mult, OP.add)
                nc.vector.tensor_tensor(
                    state16[:, col:col + 1], z0t[c, row, hc][:, C - 1:C],
                    cl[:, C - 1:C], OP.add)

            def emit_pfirst(c, row):
                pfp = ps_tiny.tile([R_, 1], f32, tag="pf")
                for hc in range(HC):
                    col = row * HC + hc
                    nc.tensor.matmul(
                        pfp[:], v16[:, hc, :], state16[:, col:col + 1],
                        start=(hc == 0), stop=(hc == HC - 1))
                nc.vector.tensor_copy(p_first[:, row:row + 1], pfp[:])

            # ---------- prologue ----------
            emit_x_dma(0)
            if NCH > 1:
                emit_x_dma(1)
            for row in range(B_core):
                emit_g(0, row)
            for hc in range(HC):
                for row in range(B_core):
                    emit_a(0, row, hc)
                for row in range(B_core):
                    emit_u(0, row, hc)

            # era A0: z0(0)+V^T(0), interposed with projections of chunk 1
            pq = build_pq(1) if NCH > 1 else []
            for hc in range(HC):
                for row in range(B_core):
                    fmm = pq.pop(0)() if pq else None
                    emit_z0_vt(0, row, hc, after=fmm)
            emit_q(0)
            while pq:
                pq.pop(0)()

            # ---------- steady eras ----------
            # era(c): Uq/cl/state of chunk c, z0/V^T of chunk c+1,
            # projections of chunk c+2 as PE filler.
            for c in range(NCH - 1):
                pq = []
                if c + 2 < NCH:
                    emit_x_dma(c + 2)
                    pq = build_pq(c + 2)
                for hc in range(HC):
                    for row in range(B_core):
                        emit_uq_cl(c, row, hc)
                        fmm = None
                        for _ in range(2):
                            if pq:
                                fmm = pq.pop(0)()
                        emit_z0_vt(c + 1, row, hc, after=fmm)
                for row in range(B_core):
                    emit_pfirst(c, row)
                emit_q(c + 1)
                while pq:
                    pq.pop(0)()

            # ---------- final era: chunk NCH-1 correction + output ----------
            cL = NCH - 1
            for row in range(B_core):
                for hc in range(HC):
                    emit_uq_cl(cL, row, hc)
                rs = slice(row * HC, (row + 1) * HC)
                nc.sync.dma_start(
                    out_d[row].rearrange("(hc p) -> p hc", p=128),
                    state16[:, rs])
    nc.finalize()
    return nc


def make_in_maps(x, Wa, ba, Wg, bg, Wu, bu, u, v, n_cores=N_CORES, C=CHUNK):
    """Shard + lay out host-side (layout transforms + fp16 casts)."""
    B_, S_, D_ = x.shape
    H_, R_ = u.shape
    KC, HC, NCH = D_ // 128, H_ // 128, S_ // C
    b_core = B_ // n_cores
    waT = np.ascontiguousarray(Wa.T).reshape(KC, 128, H_).astype(np.float16)
    wuT = np.ascontiguousarray(Wu.T).reshape(KC, 128, H_).astype(np.float16)
    wgT = np.ascontiguousarray(Wg.T).reshape(KC, 128, R_).astype(np.float16)
    uT = np.ascontiguousarray(u.T).astype(np.float16)
    vh = np.ascontiguousarray(
        v.reshape(HC, 128, R_).transpose(1, 0, 2)).astype(np.float16)
    ba_h = np.ascontiguousarray(ba.reshape(HC, 128).T).astype(np.float32)
    bu_h = np.ascontiguousarray(bu.reshape(HC, 128).T).astype(np.float32)
    bg_h = np.ascontiguousarray(bg.reshape(R_, 1)).astype(np.float32)
    in_maps = []
    for core in range(n_cores):
        rows = slice(core * b_core, (core + 1) * b_core)
        xcore = x[rows].astype(np.float16)
        xck = xcore.reshape(b_core, NCH, C, KC, 128)
        xc = np.ascontiguousarray(xck.transpose(1, 0, 3, 4, 2))
        in_maps.append({
            "xc": xc, "waT": waT, "wuT": wuT, "wgT": wgT, "uT": uT,
            "v": vh, "ba": ba_h, "bu": bu_h, "bg": bg_h,
        })
    return in_maps


def kernel(x, Wa, ba, Wg, bg, Wu, bu, u, v):
    x = np.asarray(x, dtype=np.float32)
    in_maps = make_in_maps(
        x, np.asarray(Wa), np.asarray(ba), np.asarray(Wg), np.asarray(bg),
        np.asarray(Wu), np.asarray(bu), np.asarray(u), np.asarray(v))
    nc = build_kernel()
    res = run_bass_kernel_spmd(nc, in_maps, core_ids=list(range(N_CORES)))
    return np.concatenate(
        [res.results[i]["out"].astype(np.float32) for i in range(N_CORES)],
        axis=0)


if __name__ == "__main__":
    import reference  # only when run manually next to reference.py

    inputs = {k: np.asarray(v) for k, v in reference.setup_inputs().items()}
    got = kernel(**inputs)
    exp = np.asarray(reference.reference(**inputs))
    print("relmax:", np.abs(got - exp).max() / np.abs(exp).max())
